# revision 1
# baseline (speedup 1.0000x reference)
"""Trainium2 Bass kernel: attention with rotary embedding + XL memory.

Model (B=2, T=1024, D=2048, H=16, hd=128, XL=1024):
  qkv = x @ w_qkv.T ; split q,k,v ; k_xl += pos_emb ; rope(q), rope(k)
  per head: scores = q @ [k_xl | k].T / sqrt(hd) ; softmax ; y = P @ [v_xl | v]
  out = y @ w_proj.T

Sharding: 8 cores = 2 batches x 4 head-groups (4 heads each). Each core
computes its head-group's qkv projection, rope, attention, and a partial
output projection (contraction over its 512 y-channels); the host sums the
4 partials per batch (tensor-parallel unshard) and concatenates batches.

Device design notes:
  - All matmul inputs are float32r (FP22 read) for full PE rate; tiles and
    DRAM tensors feeding matmuls are declared f32r so every producer
    (DMA or compute op) satisfies the walrus rounded-producer check.
  - GEMMs run on transposed operands (host-side layout prep) so the PE
    contraction dim is always the partition dim.
  - RoPE pair (2i, 2i+1) becomes block pair (i, 64+i) via a host-side
    permutation of W_q / W_k rows (and k_xl / pos_emb columns). The
    cross-half combine uses ACT rebase copies (tensor_tensor requires
    same start partitions; plain copies do not).
  - Scores are computed transposed (S.T = [kt, qt]) so softmax'd P.T feeds
    the AV matmul directly as rhs. Softmax skips max-subtraction (scores
    are ~N(0,1); exp is fp32-safe). The denominator is accumulated with DVE
    adds + a ones-matmul partition reduce; normalization is applied to y.
  - Since each engine executes in program order, emission is interleaved
    for overlap: all 4 heads' attention chunks round-robin (ACT exp latency
    hides under other heads' matmuls), AV matmuls trail scores by one chunk,
    and the tb1 V-GEMM / tb0 output projection are woven into attention
    chunk slots as PE gap fillers.
"""
import sys

sys.path.insert(0, "/opt/trn_rl_repo")

import numpy as np

import concourse.bass as bass  # noqa: F401
import concourse.mybir as mybir
import concourse.tile as tile
from concourse import bacc
from concourse.bass import ts
from concourse.bass_utils import run_bass_kernel_spmd  # noqa: F401 (fallback)

F32 = mybir.dt.float32
F32R = mybir.dt.float32r
AF = mybir.ActivationFunctionType
ADD = mybir.AluOpType.add

B, T, D = 2, 1024, 2048
H, HD, XL = 16, 128, 1024
HPC = 4                 # heads per core
CPB = 4                 # cores per batch
NCORES = 8
NCC = D // 128          # 16 contraction chunks
SCALE = 1.0 / np.sqrt(HD)

_CACHE: dict = {}


def _build_nc():
    nc = bacc.Bacc("TRN2", target_bir_lowering=False, debug=False)

    x_d = nc.dram_tensor("x", [2, 128, NCC, 512], F32R, kind="ExternalInput")
    wqk_d = nc.dram_tensor("wqk", [8, 128, NCC, 128], F32R, kind="ExternalInput")
    wv_d = nc.dram_tensor("wv", [2, 128, NCC, 256], F32R, kind="ExternalInput")
    cs_d = nc.dram_tensor("cs", [2, 128, T], F32, kind="ExternalInput")
    kxl_d = nc.dram_tensor("kxl", [128, 4, XL], F32R, kind="ExternalInput")
    pos_d = nc.dram_tensor("pos", [128, 4, XL], F32R, kind="ExternalInput")
    vxl_d = nc.dram_tensor("vxl", [128, 8, 512], F32R, kind="ExternalInput")
    wproj_d = nc.dram_tensor("wproj", [16, 128, 4, 128], F32R, kind="ExternalInput")
    out_d = nc.dram_tensor("out", [16, 2, 128, 512], F32, kind="ExternalOutput")

    with tile.TileContext(nc) as tc, nc.allow_low_precision(
            reason="fp32r matmul inputs: FP22 rounding is intended"):
        with (
            tc.tile_pool(name="const", bufs=1) as const,
            tc.tile_pool(name="xp", bufs=1) as xp,
            tc.tile_pool(name="wqkp", bufs=2) as wqkp,
            tc.tile_pool(name="wvp", bufs=1) as wvp,
            tc.tile_pool(name="wpp", bufs=4) as wpp,
            tc.tile_pool(name="ptp", bufs=5) as ptp,
            tc.tile_pool(name="ropep", bufs=1) as ropep,
            tc.tile_pool(name="accp", bufs=4) as accp,
            tc.tile_pool(name="smallp", bufs=1) as smallp,
            tc.tile_pool(name="outp", bufs=2) as outp,
            tc.tile_pool(name="psum", bufs=4, space="PSUM") as psum,
            tc.tile_pool(name="pyp", bufs=4, space="PSUM") as pyp,
        ):
            # ---- persistent tiles ----
            cc = const.tile([128, T], F32, tag="cc")   # [cos; cos]
            ss = const.tile([128, T], F32, tag="ss")   # [-sin; +sin]
            ones = const.tile([128, 128], F32R, tag="ones")
            qk = const.tile([128, 8, T], F32R, tag="qk")   # roped qT (0-3), kT (4-7)
            vsb = const.tile([128, 8, 512], F32R, tag="vsb")  # v, [t, d] natural
            ysb = const.tile([128, 4, T], F32R, tag="ysb")    # y.T per head

            ones_f = outp.tile([128, 128], F32, tag="ot")
            nc.vector.memset(ones_f[:], 1.0)
            nc.vector.tensor_copy(ones[:], ones_f[:])

            # ---- phase 1: QKV projection (+rope) per query t-block ----
            for tb in range(2):
                tbsl = ts(tb, 512)
                pre_wt = None
                if tb == 0:
                    pre_wt = wqkp.tile([128, NCC, 128], F32R, tag="wqk",
                                       name="wt_pre")
                    for j4 in range(4):
                        nc.sync.dma_start(pre_wt[:, 4 * j4:4 * j4 + 4, :],
                                          wqk_d[0, :, 4 * j4:4 * j4 + 4, :])
                xt = xp.tile([128, NCC, 512], F32R, tag="x")
                for j in range(8):
                    nc.sync.dma_start(xt[:, 2 * j:2 * j + 2, :],
                                      x_d[tb, :, 2 * j:2 * j + 2, :])
                if tb == 0:
                    nc.sync.dma_start(cc[:], cs_d[0])
                    nc.sync.dma_start(ss[:], cs_d[1])
                # q/k in transposed layout [d, t], fused rope out of PSUM
                for f in range(8):
                    if tb == 0 and f == 0:
                        wt = pre_wt
                    else:
                        wt = wqkp.tile([128, NCC, 128], F32R, tag="wqk")
                        for j4 in range(4):
                            nc.sync.dma_start(wt[:, 4 * j4:4 * j4 + 4, :],
                                              wqk_d[f, :, 4 * j4:4 * j4 + 4, :])
                    pmm = psum.tile([128, 512], F32, tag="ps")
                    for ci in range(NCC):
                        nc.tensor.matmul(pmm[:], wt[:, ci, :], xt[:, ci, :],
                                         start=(ci == 0), stop=(ci == NCC - 1))
                    # packed rope: new = P*[cos;cos] + swap(P)*[-sin;+sin]
                    # (swap via ACT rebase copies; TT ops need same bases)
                    sw = ropep.tile([128, 512], F32, tag="sw")
                    nc.scalar.copy(sw[0:64, :], pmm[64:128, :])
                    nc.scalar.copy(sw[64:128, :], pmm[0:64, :])
                    dst = qk[:, f, tbsl]
                    t2 = ropep.tile([128, 512], F32, tag="t2")
                    nc.vector.tensor_mul(dst, pmm[:], cc[:, tbsl])
                    nc.vector.tensor_mul(t2[:], sw[:], ss[:, tbsl])
                    nc.vector.tensor_add(dst, dst, t2[:])
                # v in natural layout [t, d]; tb1's v is deferred into the
                # attention-tb0 chunk slots (PE gap filler)
                def emit_v_group(tb, half, tt, wvt, xt=xt):
                    pv = psum.tile([128, 256], F32, tag="ps", name="pv")
                    for ci in range(NCC):
                        nc.tensor.matmul(pv[:], xt[:, ci, ts(tt, 128)],
                                         wvt[:, ci, :],
                                         start=(ci == 0), stop=(ci == NCC - 1))
                    nc.scalar.copy(vsb[:, tb * 4 + tt, ts(half, 256)], pv[:])

                if tb == 0:
                    for half in range(2):
                        wvt = wvp.tile([128, NCC, 256], F32R, tag="wv")
                        for j in range(8):
                            nc.sync.dma_start(wvt[:, 2 * j:2 * j + 2, :],
                                              wv_d[half, :, 2 * j:2 * j + 2, :])
                        for tt in range(4):
                            emit_v_group(0, half, tt, wvt)
                else:
                    v_fillers = []
                    for half in range(2):
                        def load_wv(half=half):
                            wvt = wvp.tile([128, NCC, 256], F32R, tag="wv",
                                           name=f"wvt1_{half}")
                            for j in range(8):
                                nc.sync.dma_start(
                                    wvt[:, 2 * j:2 * j + 2, :],
                                    wv_d[half, :, 2 * j:2 * j + 2, :])
                            return wvt
                        for tt in range(4):
                            def filler(half=half, tt=tt, load_wv=load_wv,
                                       xt=xt):
                                if tt == 0:
                                    filler.wvt = load_wv()
                                emit_v_group(1, half, tt, filler.wvt, xt)
                            v_fillers.append(filler)

            # ---- XL memory: load once (kxl gets pos added via accum-DMA) ----
            kxl = const.tile([128, 4, XL], F32R, tag="kxl")
            vxl = const.tile([128, 8, 512], F32R, tag="vxl")
            for j in range(4):
                nc.sync.dma_start(kxl[:, j, 0:512], kxl_d[:, j, 0:512])
                nc.sync.dma_start(kxl[:, j, 512:1024], kxl_d[:, j, 512:1024])
            for j in range(4):
                nc.gpsimd.dma_start(kxl[:, j, :], pos_d[:, j, :], accum_op=ADD)
            for j in range(8):
                nc.sync.dma_start(vxl[:, j, :], vxl_d[:, j, :])

            # ---- phase 2: attention + projection, interleaved ----
            def emit_proj(ob, tb):
                wpt = wpp.tile([128, 4, 128], F32R, tag="wp")
                nc.sync.dma_start(wpt[:, 0:2, :], wproj_d[ob, :, 0:2, :])
                nc.sync.dma_start(wpt[:, 2:4, :], wproj_d[ob, :, 2:4, :])
                po = psum.tile([128, 512], F32, tag="ps")
                for yc in range(4):
                    nc.tensor.matmul(po[:], wpt[:, yc, :],
                                     ysb[:, yc, ts(tb, 512)],
                                     start=(yc == 0), stop=(yc == 3))
                ot = outp.tile([128, 512], F32, tag="ot")
                nc.vector.tensor_copy(ot[:], po[:])
                nc.sync.dma_start(out_d[ob, tb], ot[:])

            def attn_quad(tb, fillers, every=2):
                """Chunk-interleaved attention for all 4 heads; `fillers`
                are callables emitted inside chunk slots (PE gap fillers:
                deferred v-GEMM groups or projection blocks)."""
                tbsl = ts(tb, 512)
                py, acc = {}, {}
                for h in range(4):
                    py[h] = pyp.tile([128, 512], F32, tag="py", name=f"py{h}")
                    acc[h] = accp.tile([128, 512], F32R, tag="acc",
                                       name=f"acc{h}")
                fill = list(fillers)
                pend = {}      # (h -> (pt, lv, kc)) av deferred by one chunk
                def emit_av(h):
                    pt_, lv_, kc_ = pend.pop(h)
                    nc.tensor.matmul(py[h][:], lv_, pt_[:],
                                     start=(kc_ == 0), stop=(kc_ == 15))
                for kc in range(16):
                    for h in range(4):
                        if kc < 8:
                            lk = kxl[:, h, ts(kc, 128)]
                            lv = vxl[:, kc, ts(h, 128)]
                        else:
                            lk = qk[:, 4 + h, ts(kc - 8, 128)]
                            lv = vsb[:, kc - 8, ts(h, 128)]
                        pss = psum.tile([128, 512], F32, tag="ps")
                        nc.tensor.matmul(pss[:], lk, qk[:, h, tbsl],
                                         start=True, stop=True)
                        pt = ptp.tile([128, 512], F32R, tag="pt")
                        nc.scalar.activation(pt[:], pss[:], AF.Exp, scale=SCALE)
                        if kc == 0:
                            nc.vector.tensor_copy(acc[h][:], pt[:])
                        else:
                            nc.vector.tensor_add(acc[h][:], acc[h][:], pt[:])
                        if h in pend:
                            emit_av(h)
                        pend[h] = (pt, lv, kc)
                    if kc % every == every - 1 and fill:
                        fill.pop(0)()
                for h in range(4):
                    emit_av(h)
                for h in range(4):
                    pden_t = psum.tile([128, 512], F32, tag="ps")
                    pden = pden_t[0:1, :]
                    nc.tensor.matmul(pden, ones[:, 0:1], acc[h][:],
                                     start=True, stop=True)
                    rec = smallp.tile([1, 512], F32R, tag="rec")
                    nc.vector.reciprocal(rec[:], pden)
                    pbc = psum.tile([128, 512], F32, tag="ps")
                    nc.tensor.matmul(pbc[:], ones[0:1, :], rec[:],
                                     start=True, stop=True)
                    rbc = smallp.tile([128, 512], F32, tag="rbc")
                    nc.scalar.copy(rbc[:], pbc[:])
                    nc.vector.tensor_mul(ysb[:, h, tbsl], py[h][:], rbc[:])
                while fill:
                    fill.pop(0)()

            # v-tb1 group g must precede the first av that reads vsb chunk
            # g (kc=8+g, emitted at iteration 9+g due to the av stagger);
            # slot 2g+1 <= 9+g holds for g <= 7.
            attn_quad(0, v_fillers, every=2)
            attn_quad(1, [lambda ob=ob: emit_proj(ob, 0) for ob in range(16)],
                      every=1)
            for ob in range(16):                  # proj tb1
                emit_proj(ob, 1)


    nc.compile()
    return nc


def _get_nc():
    if "nc" not in _CACHE:
        _CACHE["nc"] = _build_nc()
    return _CACHE["nc"]


_PERM = np.concatenate([np.arange(0, HD, 2), np.arange(1, HD, 2)])
_PP = np.concatenate([_PERM + i * HD for i in range(HPC)])  # per-head-block perm


def make_in_maps(x, cos, sin, k_xl, v_xl, pos_emb, w_qkv, w_proj):
    """Host-side shard + layout prep: one input dict per core."""
    x = np.asarray(x, np.float32)
    cos = np.asarray(cos, np.float32)
    sin = np.asarray(sin, np.float32)
    k_xl = np.asarray(k_xl, np.float32)
    v_xl = np.asarray(v_xl, np.float32)
    pos_emb = np.asarray(pos_emb, np.float32)
    w_qkv = np.asarray(w_qkv, np.float32)
    w_proj = np.asarray(w_proj, np.float32)

    # cs[0] = [cos; cos] ; cs[1] = [-sin; +sin]  (packed-rope factors)
    cs = np.ascontiguousarray(np.stack([
        np.concatenate([cos.T, cos.T], axis=0),
        np.concatenate([-sin.T, sin.T], axis=0),
    ]))

    in_maps = []
    for c in range(NCORES):
        b, g = c // CPB, c % CPB
        h0 = g * HPC
        cols = slice(h0 * HD, (h0 + HPC) * HD)

        # x: [tb, pi, po, tl]
        x_arr = np.ascontiguousarray(
            x[b].T.reshape(NCC, 128, 2, 512).transpose(2, 1, 0, 3))
        # w_q/w_k rows for this head group, rope-permuted; [f, pi, ci, fcol]
        wq = w_qkv[0 * D + h0 * HD:0 * D + (h0 + HPC) * HD][_PP]
        wk = w_qkv[1 * D + h0 * HD:1 * D + (h0 + HPC) * HD][_PP]
        wqk_rows = np.concatenate([wq, wk], axis=0)  # [1024, D]
        wqk_arr = np.ascontiguousarray(
            wqk_rows.reshape(8, 128, NCC, 128).transpose(0, 3, 2, 1))
        # w_v rows (unpermuted); [half, pi, ci, col]
        wv_rows = w_qkv[2 * D + h0 * HD:2 * D + (h0 + HPC) * HD]  # [512, D]
        wv_arr = np.ascontiguousarray(
            wv_rows.reshape(2, 256, NCC, 128).transpose(0, 3, 2, 1))
        # k_xl / pos_emb: permuted cols, transposed; [pi, j, t]
        kxlT = k_xl[b][:, cols][:, _PP].T  # [512, XL]
        kxl_arr = np.ascontiguousarray(
            kxlT.reshape(4, 128, XL).transpose(1, 0, 2))
        posT = pos_emb[:, cols][:, _PP].T
        pos_arr = np.ascontiguousarray(
            posT.reshape(4, 128, XL).transpose(1, 0, 2))
        # v_xl natural; [pi, j, col]
        vxl_arr = np.ascontiguousarray(
            v_xl[b][:, cols].reshape(8, 128, 512).transpose(1, 0, 2))
        # w_proj column block, transposed; [ob, pi, yc, ocol]
        wprojT = w_proj[:, cols].T  # [512, D]
        wproj_arr = np.ascontiguousarray(
            wprojT.reshape(4, 128, 16, 128).transpose(2, 1, 0, 3))

        in_maps.append({
            "x": x_arr, "wqk": wqk_arr, "wv": wv_arr, "cs": cs,
            "kxl": kxl_arr, "pos": pos_arr, "vxl": vxl_arr,
            "wproj": wproj_arr,
        })
    return in_maps


def unshard(results):
    """results: list of 8 dicts with 'out' [16, 2, 128, 512] -> [B, T, D]."""
    out = np.zeros((B, T, D), np.float32)
    for c in range(NCORES):
        b = c // CPB
        outT = np.asarray(results[c]["out"]).transpose(0, 2, 1, 3).reshape(D, T)
        out[b] += outT.T
    return out


def _get_runner():
    """Persistent jitted 8-core executable (avoids per-call retrace of the
    bass2jax lowering; the NEFF itself is cached by neuronx-cc)."""
    if "runner" in _CACHE:
        return _CACHE["runner"]
    import jax
    import jax.numpy as jnp
    from jax.sharding import Mesh, PartitionSpec, NamedSharding
    from jax.experimental.shard_map import shard_map
    from concourse.bass2jax import (_bass_exec_p, partition_id_tensor,
                                    install_neuronx_cc_hook)

    nc = _get_nc()
    install_neuronx_cc_hook()
    in_names, out_names, out_avals, zero_shapes = [], [], [], []
    for alloc in nc.m.functions[0].allocations:
        if not isinstance(alloc, mybir.MemoryLocationSet):
            continue
        name = alloc.memorylocations[0].name
        if alloc.kind == "ExternalInput":
            if nc.partition_id_tensor is None or \
                    name != nc.partition_id_tensor.name:
                in_names.append(name)
        elif alloc.kind == "ExternalOutput":
            shape = tuple(alloc.tensor_shape)
            np_dt = mybir.dt.np(alloc.dtype)
            out_names.append(name)
            out_avals.append(jax.core.ShapedArray(shape, np_dt))
            zero_shapes.append((shape, np_dt))
    n_params, n_outs = len(in_names), len(out_names)
    all_in = in_names + out_names
    if nc.partition_id_tensor is not None:
        all_in = all_in + [nc.partition_id_tensor.name]

    def _body(*args):
        operands = list(args)
        if nc.partition_id_tensor is not None:
            operands.append(partition_id_tensor())
        return tuple(_bass_exec_p.bind(
            *operands, out_avals=tuple(out_avals), in_names=tuple(all_in),
            out_names=tuple(out_names), lowering_input_output_aliases=(),
            sim_require_finite=True, sim_require_nnan=True, nc=nc))

    devices = jax.devices()[:NCORES]
    mesh = Mesh(np.asarray(devices), ("core",))
    fn = jax.jit(
        shard_map(_body, mesh=mesh,
                  in_specs=(PartitionSpec("core"),) * (n_params + n_outs),
                  out_specs=(PartitionSpec("core"),) * n_outs,
                  check_rep=False),
        donate_argnums=tuple(range(n_params, n_params + n_outs)),
        keep_unused=True)
    sharding = NamedSharding(mesh, PartitionSpec("core"))
    zfn = jax.jit(
        lambda: tuple(jnp.zeros((NCORES * s[0], *s[1:]), d)
                      for s, d in zero_shapes),
        out_shardings=(sharding,) * n_outs)
    runner = (fn, zfn, in_names, out_names, out_avals, sharding)
    _CACHE["runner"] = runner
    return runner


def kernel(x, cos, sin, k_xl, v_xl, pos_emb, w_qkv, w_proj, is_causal=0,
           **_ignored):
    # is_causal is 0 for this problem spec (fill=arange, shape []); the
    # non-causal path is the only one implemented.
    import jax
    in_maps = make_in_maps(x, cos, sin, k_xl, v_xl, pos_emb, w_qkv, w_proj)
    fn, zfn, in_names, out_names, out_avals, sharding = _get_runner()
    concat_in = [
        jax.device_put(
            np.concatenate([in_maps[c][nm] for c in range(NCORES)], axis=0),
            sharding)
        for nm in in_names]
    outs = fn(*concat_in, *zfn())
    results = [
        {nm: np.asarray(outs[i]).reshape(NCORES, *out_avals[i].shape)[c]
         for i, nm in enumerate(out_names)}
        for c in range(NCORES)]
    _CACHE["last_results"] = None
    return unshard(results)



# revision 22
# speedup vs baseline: 1.2763x; 1.2763x over previous
"""Trainium2 Bass kernel: attention with rotary embedding + XL memory.

Model (B=2, T=1024, D=2048, H=16, hd=128, XL=1024):
  qkv = x @ w_qkv.T ; split q,k,v ; k_xl += pos_emb ; rope(q), rope(k)
  per head: scores = q @ [k_xl | k].T / sqrt(hd) ; softmax ; y = P @ [v_xl | v]
  out = y @ w_proj.T
is_causal is 0 for this problem spec; only the non-causal path exists.

Sharding: 8 cores = 2 batches x 4 head-groups (4 heads each). Each core
computes its head-group's qkv projection, rope, attention, and a partial
output projection (contraction over its 512 y-channels); the host sums the
4 partials per batch (tensor-parallel unshard) and concatenates batches.

Device design notes:
  - All matmul inputs are bf16 (same PE rate as fp32r for moving>=256 but
    half the DMA bytes and 2x DVE element-wise rate); PSUM accumulation is
    fp32. k_xl+pos_emb is fused on the host, as are all transposes, the
    rope pair permutation (pair (2i,2i+1) -> blocks (i, 64+i)) and bf16
    rounding. Weights are loaded once (f-outer loops); w_proj is resident.
  - GEMMs contract over the partition dim; every matmul keeps the moving
    (output free) dim at 512 for the 1 cycle/row rate.
  - Scores are computed transposed (S.T = [kt, qt]) so softmax'd P.T feeds
    the AV matmul directly as rhs. Softmax skips max-subtraction (scores
    are ~N(0,1)). The denominator is 4 rows of one PSUM tile via ones-
    matmuls, one 4-row reciprocal, and ones-broadcast matmuls; the
    normalization is applied to y.
  - Schedule (single PE program, ACT/DVE trail): wavefront q-GEMMs while
    x streams chunk-by-chunk (PE starts ~2us in), k-GEMMs + v chunks 0-3,
    then two attention quads. Attention is ACT-paced (4 exps/chunk-slot
    ~2.45us vs 1.7us of PE scores+AV), so PE fill work is woven into the
    slots: v chunks 4-7 in quad tb0, tb0 projection blocks in quad tb1,
    den stages at quad boundaries. Projection tail is PE-bound with the
    resident w_proj. PSUM = two 4-bank rings (py accumulators + rotating
    score/GEMM/den tiles).
"""
import sys

sys.path.insert(0, "/opt/trn_rl_repo")

import numpy as np

import concourse.bass as bass  # noqa: F401
import concourse.mybir as mybir
import concourse.tile as tile
from concourse import bacc
from concourse.bass import ts
from concourse.bass_utils import run_bass_kernel_spmd  # noqa: F401 (fallback)

F32 = mybir.dt.float32
BF16 = mybir.dt.bfloat16
AF = mybir.ActivationFunctionType

B, T, D = 2, 1024, 2048
H, HD, XL = 16, 128, 1024
HPC = 4                 # heads per core
CPB = 4                 # cores per batch
NCORES = 8
NCC = D // 128          # 16 contraction chunks
SCALE = 1.0 / np.sqrt(HD)

_CACHE: dict = {}


def _build_nc():
    nc = bacc.Bacc("TRN2", target_bir_lowering=False, debug=False)

    x_d = nc.dram_tensor("x", [128, NCC, T], BF16, kind="ExternalInput")
    wqk_d = nc.dram_tensor("wqk", [8, 128, NCC, 128], BF16,
                           kind="ExternalInput")
    wv_d = nc.dram_tensor("wv", [128, NCC, 512], BF16, kind="ExternalInput")
    cs_d = nc.dram_tensor("cs", [2, 128, T], F32, kind="ExternalInput")
    kxl_d = nc.dram_tensor("kxl", [128, 4, XL], BF16, kind="ExternalInput")
    vxl_d = nc.dram_tensor("vxl", [128, 8, 512], BF16, kind="ExternalInput")
    wproj_d = nc.dram_tensor("wproj", [128, 16, 4, 128], BF16,
                             kind="ExternalInput")
    out_d = nc.dram_tensor("out", [2, 128, 16, 512], BF16,
                           kind="ExternalOutput")

    with tile.TileContext(nc) as tc, nc.allow_low_precision(
            reason="bf16 matmul inputs: rounding is intended"):
        with (
            tc.tile_pool(name="const", bufs=1) as const,
            tc.tile_pool(name="wqkBp", bufs=4) as wqkBp,
            tc.tile_pool(name="swp", bufs=2) as swp,
            tc.tile_pool(name="t2p", bufs=2) as t2p,
            tc.tile_pool(name="ptp", bufs=10) as ptp,
            tc.tile_pool(name="accp", bufs=8) as accp,
            tc.tile_pool(name="smallp", bufs=4) as smallp,
            tc.tile_pool(name="outp", bufs=8) as outp,
            tc.tile_pool(name="out4p", bufs=2) as out4p,
            tc.tile_pool(name="pyp", bufs=4, space="PSUM") as pyp,
            tc.tile_pool(name="scp", bufs=4, space="PSUM") as scp,
        ):
            # ---- persistent tiles ----
            cc = const.tile([128, T], F32, tag="cc")   # [cos; cos]
            ss = const.tile([128, T], F32, tag="ss")   # [-sin; +sin]
            ones = const.tile([128, 128], BF16, tag="ones")
            qk = const.tile([128, 8, T], BF16, tag="qk")  # q.T (0-3), k.T (4-7)
            vsb = const.tile([128, 8, 512], BF16, tag="vsb")  # v [t, d] natural
            ysb = const.tile([128, 4, T], BF16, tag="ysb")    # y.T per head
            kxl = const.tile([128, 4, XL], BF16, tag="kxl")
            vxl = const.tile([128, 8, 512], BF16, tag="vxl")
            wpall = const.tile([128, 16, 4, 128], BF16, tag="wp")
            xt = const.tile([128, NCC, T], BF16, tag="x")
            wvt = const.tile([128, NCC, 512], BF16, tag="wv")
            wqkA = const.tile([128, 4, NCC, 128], BF16, tag="wqkA")

            nc.vector.memset(ones[:], 1.0)

            # ---- DMA queue, priority order: one serial DMA resource, so
            # interleave x chunks (which gate the first matmuls) with the
            # wqkA blocks; cc/ss land before the first rope at x-end ----
            def dma_wA(f, half):
                nc.sync.dma_start(wqkA[:, f, ts(half, 8), :],
                                  wqk_d[f, :, ts(half, 8), :])
            dma_wA(0, 0)
            nc.sync.dma_start(xt[:, 0], x_d[:, 0])
            dma_wA(1, 0)
            nc.sync.dma_start(xt[:, 1], x_d[:, 1])
            nc.sync.dma_start(xt[:, 2], x_d[:, 2])
            dma_wA(2, 0)
            nc.sync.dma_start(xt[:, 3], x_d[:, 3])
            nc.sync.dma_start(cc[:], cs_d[0])
            nc.sync.dma_start(xt[:, 4], x_d[:, 4])
            dma_wA(3, 0)
            nc.sync.dma_start(xt[:, 5], x_d[:, 5])
            dma_wA(0, 1)
            nc.sync.dma_start(xt[:, 6], x_d[:, 6])
            dma_wA(1, 1)
            nc.sync.dma_start(xt[:, 7], x_d[:, 7])
            nc.sync.dma_start(ss[:], cs_d[1])
            nc.sync.dma_start(xt[:, 8], x_d[:, 8])
            dma_wA(2, 1)
            nc.sync.dma_start(xt[:, 9], x_d[:, 9])
            dma_wA(3, 1)
            for ci in range(10, NCC):
                nc.sync.dma_start(xt[:, ci], x_d[:, ci])
            wqkB = []
            for f in range(4):
                wt = wqkBp.tile([128, NCC, 128], BF16, tag="wqkB",
                                name=f"wqkB{f}")
                nc.sync.dma_start(wt[:], wqk_d[4 + f])
                wqkB.append(wt)
            nc.sync.dma_start(wvt[:], wv_d[:])
            nc.sync.dma_start(kxl[:], kxl_d[:])
            nc.sync.dma_start(vxl[:], vxl_d[:])
            nc.sync.dma_start(wpall[:], wproj_d[:])

            # preload the Exp activation table during the x stream
            warm = smallp.tile([1, 16], BF16, tag="warm")
            nc.scalar.activation(warm[:], ones[0:1, 0:16], AF.Exp, scale=1.0)

            def rope(psrc, fslot, tbsl):
                """qk[:, fslot, tbsl] = psrc*[cos;cos] + swap(psrc)*[-sin;sin]
                (swap via ACT rebase copies; tensor ops need same bases)."""
                sw = swp.tile([128, 512], F32, tag="sw")
                nc.scalar.copy(sw[0:64, :], psrc[64:128, :])
                nc.scalar.copy(sw[64:128, :], psrc[0:64, :])
                dst = qk[:, fslot, tbsl]
                t2 = t2p.tile([128, 512], BF16, tag="t2")
                nc.vector.tensor_mul(dst, psrc[:], cc[:, tbsl])
                nc.vector.tensor_mul(t2[:], sw[:], ss[:, tbsl])
                nc.vector.tensor_add(dst, dst, t2[:])

            # ---- phase A: q-GEMMs (f0-3 x tb0/tb1 minus (3,tb1)), ci-
            # wavefront over the streaming x so PE starts as soon as chunk 0
            # + w f0 land. Only 7 accumulators so the scp ring keeps a free
            # slot and the follow-on GEMMs never wait on the rope drain. ----
            qp = {}
            for fi in range(4):
                for tb in range(2):
                    if (fi, tb) == (3, 1):
                        continue
                    pool = pyp if fi < 2 else scp
                    qp[fi, tb] = pool.tile([128, 512], F32,
                                           tag="py" if fi < 2 else "sc",
                                           name=f"qA{fi}{tb}")
            skew = {0: 0, 1: 1, 2: 2, 3: 8}
            for w in range(NCC + 8):
                for fi in range(4):
                    ci = w - skew[fi]
                    if not (0 <= ci < NCC):
                        continue
                    for tb in range(2):
                        if (fi, tb) == (3, 1):
                            continue
                        nc.tensor.matmul(qp[fi, tb][:], wqkA[:, fi, ci, :],
                                         xt[:, ci, ts(tb, 512)],
                                         start=(ci == 0), stop=(ci == NCC - 1))

            # ---- phase A2: remaining q-GEMM, k-GEMMs (f4-7), v chunks 0-3,
            # with ropes interleaved so each scp ring slot is freed (rope
            # read) at least 4 allocations before it is reused ----
            def gemm_qk(wt_ap, tb, name):
                pk = scp.tile([128, 512], F32, tag="sc", name=name)
                for ci in range(NCC):
                    nc.tensor.matmul(pk[:], wt_ap[:, ci, :],
                                     xt[:, ci, ts(tb, 512)],
                                     start=(ci == 0), stop=(ci == NCC - 1))
                return pk

            def emit_v(tt, use_act):
                pv = scp.tile([128, 512], F32, tag="sc", name=f"pv{tt}")
                for ci in range(NCC):
                    nc.tensor.matmul(pv[:], xt[:, ci, ts(tt, 128)],
                                     wvt[:, ci, :],
                                     start=(ci == 0), stop=(ci == NCC - 1))
                if use_act:
                    nc.scalar.copy(vsb[:, tt, :], pv[:])
                else:
                    nc.vector.tensor_copy(vsb[:, tt, :], pv[:])

            rope(qp[2, 0], 2, ts(0, 512))
            pk31 = gemm_qk(wqkA[:, 3], "qA31")
            rope(qp[2, 1], 2, ts(1, 512))
            rope(qp[3, 0], 3, ts(0, 512))
            pk = {}
            pk[4, 0] = gemm_qk(wqkB[0], "k40")
            rope(pk31, 3, ts(1, 512))
            pk[4, 1] = gemm_qk(wqkB[0], "k41")
            rope(pk[4, 0], 4, ts(0, 512))
            pk[5, 0] = gemm_qk(wqkB[1], "k50")
            rope(pk[4, 1], 4, ts(1, 512))
            pk[5, 1] = gemm_qk(wqkB[1], "k51")
            rope(pk[5, 0], 5, ts(0, 512))
            pk[6, 0] = gemm_qk(wqkB[2], "k60")
            rope(pk[5, 1], 5, ts(1, 512))
            pk[6, 1] = gemm_qk(wqkB[2], "k61")
            rope(pk[6, 0], 6, ts(0, 512))
            pk[7, 0] = gemm_qk(wqkB[3], "k70")
            rope(pk[6, 1], 6, ts(1, 512))
            pk[7, 1] = gemm_qk(wqkB[3], "k71")
            rope(pk[7, 0], 7, ts(0, 512))
            emit_v(0, use_act=True)
            rope(pk[7, 1], 7, ts(1, 512))
            emit_v(1, use_act=True)
            rope(qp[0, 0], 0, ts(0, 512))
            rope(qp[0, 1], 0, ts(1, 512))
            emit_v(2, use_act=True)
            rope(qp[1, 0], 1, ts(0, 512))
            rope(qp[1, 1], 1, ts(1, 512))
            emit_v(3, use_act=True)

            # ---- attention quads + interleaved den / projection ----
            OFF = {0: 0, 1: 0, 2: 2, 3: 2}   # head slot stagger

            def emit_quad(tb, fillers, spacing=1, fill_from=1, den_cb=None):
                """Chunk-interleaved attention for 4 heads over 16 k-chunks.
                Heads 2,3 run two slots behind heads 0,1 so this quad's den
                work for heads 0,1 (den_cb stages "s1a"/"s3a") overlaps the
                staggered tail instead of serializing at the quad boundary.
                fillers are PE work callables popped per slot."""
                tbsl = ts(tb, 512)
                acc, py = {}, {}
                for h in range(4):
                    py[h] = pyp.tile([128, 512], F32, tag="py",
                                     name=f"py{tb}{h}")
                    acc[h] = accp.tile([128, 512], BF16, tag="acc",
                                       name=f"acc{tb}{h}")
                fill = list(fillers)
                pend = {}      # h -> (pt, lv, kc); av deferred by one chunk

                def emit_av(h):
                    pt_, lv_, kc_ = pend.pop(h)
                    nc.tensor.matmul(py[h][:], lv_, pt_[:],
                                     start=(kc_ == 0), stop=(kc_ == 15))
                for s in range(18):
                    for h in range(4):
                        kc = s - OFF[h]
                        if kc == 16:
                            emit_av(h)
                        if not (0 <= kc < 16):
                            continue
                        if kc < 8:
                            lk = kxl[:, h, ts(kc, 128)]
                            lv = vxl[:, kc, ts(h, 128)]
                        else:
                            lk = qk[:, 4 + h, ts(kc - 8, 128)]
                            lv = vsb[:, kc - 8, ts(h, 128)]
                        pss = scp.tile([128, 512], F32, tag="sc")
                        nc.tensor.matmul(pss[:], lk, qk[:, h, tbsl],
                                         start=True, stop=True)
                        pt = ptp.tile([128, 512], BF16, tag="pt")
                        nc.scalar.activation(pt[:], pss[:], AF.Exp,
                                             scale=SCALE)
                        if kc == 0:
                            nc.vector.tensor_copy(acc[h][:], pt[:])
                        else:
                            nc.vector.tensor_add(acc[h][:], acc[h][:], pt[:])
                        if h in pend:
                            emit_av(h)
                        pend[h] = (pt, lv, kc)
                    if s == 16 and den_cb is not None:
                        den_cb("s1a", acc, py)
                    if s == 17 and den_cb is not None:
                        den_cb("s3a", acc, py)
                    if fill and s >= fill_from and \
                            (s - fill_from) % spacing == 0:
                        fill.pop(0)()
                for h in range(4):
                    if h in pend:
                        emit_av(h)
                while fill:
                    fill.pop(0)()
                return acc, py

            def den_s1(acc, heads):
                """Per-head partition-sum of acc + reciprocal -> [1,512]."""
                recs = {}
                for h in heads:
                    pden = scp.tile([128, 512], F32, tag="sc", name="pden")
                    nc.tensor.matmul(pden[0:1, :], ones[:, 0:1],
                                     acc[h][:], start=True, stop=True)
                    rec = smallp.tile([1, 512], BF16, tag="rec")
                    nc.vector.reciprocal(rec[:], pden[0:1, :])
                    recs[h] = rec
                return recs

            def den_s3(recs, py, tb, heads):
                tbsl = ts(tb, 512)
                for h in heads:
                    pbc = scp.tile([128, 512], F32, tag="sc", name="pbc")
                    nc.tensor.matmul(pbc[:], ones[0:1, :], recs[h][:],
                                     start=True, stop=True)
                    rbc = smallp.tile([128, 512], F32, tag="rbc")
                    nc.vector.tensor_copy(rbc[:], pbc[:])
                    nc.vector.tensor_mul(ysb[:, h, tbsl], py[h][:], rbc[:])

            def emit_proj(ob, tb, pool=None, stage=None, parity=[0]):
                pool = pool or scp
                po = pool.tile([128, 512], F32,
                               tag="sc" if pool is scp else "py",
                               name=f"po{ob}{tb}")
                for yc in range(4):
                    nc.tensor.matmul(po[:], wpall[:, ob, yc, :],
                                     ysb[:, yc, ts(tb, 512)],
                                     start=(yc == 0), stop=(yc == 3))
                if stage is None:
                    ot = outp.tile([128, 512], BF16, tag="ot")
                    dst = ot[:]
                else:
                    st, j = stage
                    dst = st[:, j, :]
                if parity[0] % 2:
                    nc.scalar.copy(dst, po[:])
                else:
                    nc.vector.tensor_copy(dst, po[:])
                parity[0] += 1
                if stage is None:
                    nc.sync.dma_start(out_d[tb, :, ob, :], ot[:])

            # quad tb0: fill with v chunks 4-7 (each ready well before
            # its first AV read); den for heads 0,1 overlaps the stagger tail
            hold = {}

            def den0_cb(stage, acc, py):
                if stage == "s1a":
                    hold["r01"] = den_s1(acc, (0, 1))
                else:
                    den_s3(hold.pop("r01"), py, 0, (0, 1))

            acc_d, py_d = {}, {}
            acc_d[0], py_d[0] = emit_quad(
                0, [lambda tt=tt: emit_v(tt, use_act=False)
                    for tt in range(4, 8)], spacing=3, den_cb=den0_cb)

            # tb0 heads 2,3 den + tb0 projections fill quad tb1
            def f_s1b():
                hold["r23"] = den_s1(acc_d[0], (2, 3))

            def f_s3b():
                den_s3(hold.pop("r23"), py_d[0], 0, (2, 3))

            def den1_cb(stage, acc, py):
                if stage == "s1a":
                    hold["r01b"] = den_s1(acc, (0, 1))
                else:
                    den_s3(hold.pop("r01b"), py, 1, (0, 1))

            q1_fill = [f_s1b, f_s3b]
            q1_fill += [lambda ob=ob: emit_proj(ob, 0) for ob in range(12)]
            acc_d[1], py_d[1] = emit_quad(1, q1_fill, fill_from=0,
                                          den_cb=den1_cb)

            # tail: tb1 heads 2,3 den + remaining projections (w_proj
            # resident); po tiles alternate between both PSUM rings so the
            # ring-slot turnaround (mm -> sem -> copy -> free) stays off PE
            rec1 = den_s1(acc_d[1], (2, 3))
            for ob in range(12, 14):
                emit_proj(ob, 0)
            den_s3(rec1, py_d[1], 1, (2, 3))
            emit_proj(14, 0, pool=scp)
            emit_proj(15, 0, pool=pyp)
            for g in range(3):
                st = out4p.tile([128, 4, 512], BF16, tag="ot4",
                               name=f"st{g}")
                for j in range(4):
                    ob = 4 * g + j
                    emit_proj(ob, 1, pool=(scp if j % 2 == 0 else pyp),
                              stage=(st, j))
                nc.sync.dma_start(out_d[1, :, ts(g, 4), :], st[:])
            for ob in range(12, 16):
                emit_proj(ob, 1, pool=(scp if ob % 2 == 0 else pyp))

    nc.compile()
    return nc


def _get_nc():
    if "nc" not in _CACHE:
        _CACHE["nc"] = _build_nc()
    return _CACHE["nc"]


_PERM = np.concatenate([np.arange(0, HD, 2), np.arange(1, HD, 2)])
_PP = np.concatenate([_PERM + i * HD for i in range(HPC)])  # per-head-block


def make_in_maps(x, cos, sin, k_xl, v_xl, pos_emb, w_qkv, w_proj):
    """Host-side shard + layout prep: one input dict per core."""
    import ml_dtypes
    bf16 = ml_dtypes.bfloat16
    x = np.asarray(x, np.float32)
    cos = np.asarray(cos, np.float32)
    sin = np.asarray(sin, np.float32)
    k_xl = np.asarray(k_xl, np.float32) + np.asarray(pos_emb, np.float32)
    v_xl = np.asarray(v_xl, np.float32)
    w_qkv = np.asarray(w_qkv, np.float32)
    w_proj = np.asarray(w_proj, np.float32)

    # cs[0] = [cos; cos] ; cs[1] = [-sin; +sin]  (packed-rope factors)
    cs = np.ascontiguousarray(np.stack([
        np.concatenate([cos.T, cos.T], axis=0),
        np.concatenate([-sin.T, sin.T], axis=0),
    ]))

    in_maps = []
    for c in range(NCORES):
        b, g = c // CPB, c % CPB
        h0 = g * HPC
        cols = slice(h0 * HD, (h0 + HPC) * HD)

        # x: [pi, ci, t]
        x_arr = np.ascontiguousarray(
            x[b].T.reshape(NCC, 128, T).transpose(1, 0, 2)).astype(bf16)
        # w_q/w_k rows for this head group, rope-permuted; [f, pi, ci, fcol]
        wq = w_qkv[0 * D + h0 * HD:0 * D + (h0 + HPC) * HD][_PP]
        wk = w_qkv[1 * D + h0 * HD:1 * D + (h0 + HPC) * HD][_PP]
        wqk_rows = np.concatenate([wq, wk], axis=0)  # [1024, D]
        wqk_arr = np.ascontiguousarray(
            wqk_rows.reshape(8, 128, NCC, 128).transpose(0, 3, 2, 1)
        ).astype(bf16)
        # w_v rows (unpermuted); [pi, ci, col]
        wv_rows = w_qkv[2 * D + h0 * HD:2 * D + (h0 + HPC) * HD]  # [512, D]
        wv_arr = np.ascontiguousarray(
            wv_rows.T.reshape(NCC, 128, 512).transpose(1, 0, 2)).astype(bf16)
        # k_xl (pos already added): permuted cols, transposed; [pi, j, t]
        kxlT = k_xl[b][:, cols][:, _PP].T  # [512, XL]
        kxl_arr = np.ascontiguousarray(
            kxlT.reshape(4, 128, XL).transpose(1, 0, 2)).astype(bf16)
        # v_xl natural; [pi, j, col]
        vxl_arr = np.ascontiguousarray(
            v_xl[b][:, cols].reshape(8, 128, 512).transpose(1, 0, 2)
        ).astype(bf16)
        # w_proj column block, transposed; [pi, ob, yc, ocol]
        wprojT = w_proj[:, cols].T  # [512, D]
        wproj_arr = np.ascontiguousarray(
            wprojT.reshape(4, 128, 16, 128).transpose(1, 2, 0, 3)
        ).astype(bf16)

        in_maps.append({
            "x": x_arr, "wqk": wqk_arr, "wv": wv_arr, "cs": cs,
            "kxl": kxl_arr, "vxl": vxl_arr, "wproj": wproj_arr,
        })
    return in_maps


def unshard(results):
    """results: list of 8 dicts with 'out' [2, 128, 16, 512] (tb, pi, ob,
    col) -> [B, T, D]."""
    out = np.zeros((B, T, D), np.float32)
    for c in range(NCORES):
        b = c // CPB
        outT = np.asarray(results[c]["out"]).astype(np.float32)\
            .transpose(2, 1, 0, 3).reshape(D, T)
        out[b] += outT.T
    return out


def _get_runner():
    """Persistent jitted 8-core executable (avoids per-call retrace of the
    bass2jax lowering; the NEFF itself is cached by neuronx-cc)."""
    if "runner" in _CACHE:
        return _CACHE["runner"]
    import jax
    import jax.numpy as jnp
    from jax.sharding import Mesh, PartitionSpec, NamedSharding
    from jax.experimental.shard_map import shard_map
    from concourse.bass2jax import (_bass_exec_p, partition_id_tensor,
                                    install_neuronx_cc_hook)

    nc = _get_nc()
    install_neuronx_cc_hook()
    in_names, out_names, out_avals, zero_shapes = [], [], [], []
    for alloc in nc.m.functions[0].allocations:
        if not isinstance(alloc, mybir.MemoryLocationSet):
            continue
        name = alloc.memorylocations[0].name
        if alloc.kind == "ExternalInput":
            if nc.partition_id_tensor is None or \
                    name != nc.partition_id_tensor.name:
                in_names.append(name)
        elif alloc.kind == "ExternalOutput":
            shape = tuple(alloc.tensor_shape)
            np_dt = mybir.dt.np(alloc.dtype)
            out_names.append(name)
            out_avals.append(jax.core.ShapedArray(shape, np_dt))
            zero_shapes.append((shape, np_dt))
    n_params, n_outs = len(in_names), len(out_names)
    all_in = in_names + out_names
    if nc.partition_id_tensor is not None:
        all_in = all_in + [nc.partition_id_tensor.name]

    def _body(*args):
        operands = list(args)
        if nc.partition_id_tensor is not None:
            operands.append(partition_id_tensor())
        return tuple(_bass_exec_p.bind(
            *operands, out_avals=tuple(out_avals), in_names=tuple(all_in),
            out_names=tuple(out_names), lowering_input_output_aliases=(),
            sim_require_finite=True, sim_require_nnan=True, nc=nc))

    devices = jax.devices()[:NCORES]
    mesh = Mesh(np.asarray(devices), ("core",))
    fn = jax.jit(
        shard_map(_body, mesh=mesh,
                  in_specs=(PartitionSpec("core"),) * (n_params + n_outs),
                  out_specs=(PartitionSpec("core"),) * n_outs,
                  check_rep=False),
        donate_argnums=tuple(range(n_params, n_params + n_outs)),
        keep_unused=True)
    sharding = NamedSharding(mesh, PartitionSpec("core"))
    zfn = jax.jit(
        lambda: tuple(jnp.zeros((NCORES * s[0], *s[1:]), d)
                      for s, d in zero_shapes),
        out_shardings=(sharding,) * n_outs)
    runner = (fn, zfn, in_names, out_names, out_avals, sharding)
    _CACHE["runner"] = runner
    return runner


def kernel(x, cos, sin, k_xl, v_xl, pos_emb, w_qkv, w_proj, is_causal=0,
           **_ignored):
    import jax
    in_maps = make_in_maps(x, cos, sin, k_xl, v_xl, pos_emb, w_qkv, w_proj)
    fn, zfn, in_names, out_names, out_avals, sharding = _get_runner()
    concat_in = [
        jax.device_put(
            np.concatenate([in_maps[c][nm] for c in range(NCORES)], axis=0),
            sharding)
        for nm in in_names]
    outs = fn(*concat_in, *zfn())
    results = [
        {nm: np.asarray(outs[i]).reshape(NCORES, *out_avals[i].shape)[c]
         for i, nm in enumerate(out_names)}
        for c in range(NCORES)]
    _CACHE["last_results"] = None
    return unshard(results)


# revision 26
# speedup vs baseline: 1.2921x; 1.0124x over previous
"""Trainium2 Bass kernel: attention with rotary embedding + XL memory.

Model (B=2, T=1024, D=2048, H=16, hd=128, XL=1024):
  qkv = x @ w_qkv.T ; split q,k,v ; k_xl += pos_emb ; rope(q), rope(k)
  per head: scores = q @ [k_xl | k].T / sqrt(hd) ; softmax ; y = P @ [v_xl | v]
  out = y @ w_proj.T
is_causal is 0 for this problem spec; only the non-causal path exists.

Sharding: 8 cores = 2 batches x 4 head-groups (4 heads each). Each core
computes its head-group's qkv projection, rope, attention, and a partial
output projection (contraction over its 512 y-channels); the host sums the
4 partials per batch (tensor-parallel unshard) and concatenates batches.

Device design notes:
  - All matmul inputs are bf16 (same PE rate as fp32r for moving>=256 but
    half the DMA bytes and 2x DVE element-wise rate); PSUM accumulation is
    fp32. k_xl+pos_emb is fused on the host, as are all transposes, the
    rope pair permutation (pair (2i,2i+1) -> blocks (i, 64+i)) and bf16
    rounding. Weights are loaded once (f-outer loops); w_proj is resident.
  - GEMMs contract over the partition dim; every matmul keeps the moving
    (output free) dim at 512 for the 1 cycle/row rate.
  - Scores are computed transposed (S.T = [kt, qt]) so softmax'd P.T feeds
    the AV matmul directly as rhs. Softmax skips max-subtraction (scores
    are ~N(0,1)). The denominator is 4 rows of one PSUM tile via ones-
    matmuls, one 4-row reciprocal, and ones-broadcast matmuls; the
    normalization is applied to y.
  - Schedule (single PE program, ACT/DVE trail): wavefront q-GEMMs while
    x streams chunk-by-chunk (PE starts ~2us in), k-GEMMs + v chunks 0-3,
    then two attention quads. Attention is ACT-paced (4 exps/chunk-slot
    ~2.45us vs 1.7us of PE scores+AV), so PE fill work is woven into the
    slots: v chunks 4-7 in quad tb0, tb0 projection blocks in quad tb1,
    den stages at quad boundaries. Projection tail is PE-bound with the
    resident w_proj. PSUM = two 4-bank rings (py accumulators + rotating
    score/GEMM/den tiles).
"""
import sys

sys.path.insert(0, "/opt/trn_rl_repo")

import numpy as np

import concourse.bass as bass  # noqa: F401
import concourse.mybir as mybir
import concourse.tile as tile
from concourse import bacc
from concourse.bass import ts
from concourse.bass_utils import run_bass_kernel_spmd  # noqa: F401 (fallback)

F32 = mybir.dt.float32
BF16 = mybir.dt.bfloat16
AF = mybir.ActivationFunctionType

B, T, D = 2, 1024, 2048
H, HD, XL = 16, 128, 1024
HPC = 4                 # heads per core
CPB = 4                 # cores per batch
NCORES = 8
NCC = D // 128          # 16 contraction chunks
SCALE = 1.0 / np.sqrt(HD)

_CACHE: dict = {}


def _build_nc():
    nc = bacc.Bacc("TRN2", target_bir_lowering=False, debug=False)

    x_d = nc.dram_tensor("x", [128, NCC, T], BF16, kind="ExternalInput")
    wqk_d = nc.dram_tensor("wqk", [8, 128, NCC, 128], BF16,
                           kind="ExternalInput")
    wv_d = nc.dram_tensor("wv", [128, NCC, 512], BF16, kind="ExternalInput")
    cs_d = nc.dram_tensor("cs", [2, 128, T], F32, kind="ExternalInput")
    kxl_d = nc.dram_tensor("kxl", [128, 4, XL], BF16, kind="ExternalInput")
    vxl_d = nc.dram_tensor("vxl", [128, 8, 512], BF16, kind="ExternalInput")
    wproj_d = nc.dram_tensor("wproj", [128, 16, 4, 128], BF16,
                             kind="ExternalInput")
    out_d = nc.dram_tensor("out", [2, 128, 16, 512], BF16,
                           kind="ExternalOutput")

    with tile.TileContext(nc) as tc, nc.allow_low_precision(
            reason="bf16 matmul inputs: rounding is intended"):
        with (
            tc.tile_pool(name="const", bufs=1) as const,
            tc.tile_pool(name="wqkBp", bufs=4) as wqkBp,
            tc.tile_pool(name="swp", bufs=2) as swp,
            tc.tile_pool(name="t2p", bufs=2) as t2p,
            tc.tile_pool(name="ptp", bufs=10) as ptp,
            tc.tile_pool(name="accp", bufs=8) as accp,
            tc.tile_pool(name="smallp", bufs=4) as smallp,
            tc.tile_pool(name="outp", bufs=8) as outp,
            tc.tile_pool(name="out4p", bufs=2) as out4p,
            tc.tile_pool(name="pyp", bufs=4, space="PSUM") as pyp,
            tc.tile_pool(name="scp", bufs=4, space="PSUM") as scp,
        ):
            # ---- persistent tiles ----
            cc = const.tile([128, T], F32, tag="cc")   # [cos; cos]
            ss = const.tile([128, T], F32, tag="ss")   # [-sin; +sin]
            ones = const.tile([128, 128], BF16, tag="ones")
            qk = const.tile([128, 8, T], BF16, tag="qk")  # q.T (0-3), k.T (4-7)
            vsb = const.tile([128, 8, 512], BF16, tag="vsb")  # v [t, d] natural
            ysb = const.tile([128, 4, T], BF16, tag="ysb")    # y.T per head
            kxl = const.tile([128, 4, XL], BF16, tag="kxl")
            vxl = const.tile([128, 8, 512], BF16, tag="vxl")
            wpall = const.tile([128, 16, 4, 128], BF16, tag="wp")
            xt = const.tile([128, NCC, T], BF16, tag="x")
            wvt = const.tile([128, NCC, 512], BF16, tag="wv")
            wqkA = const.tile([128, 4, NCC, 128], BF16, tag="wqkA")

            nc.vector.memset(ones[:], 1.0)

            # ---- DMA queue, priority order: one serial DMA resource, so
            # interleave x chunks (which gate the first matmuls) with the
            # wqkA blocks; cc/ss land before the first rope at x-end ----
            def dma_wA(f, half):
                nc.sync.dma_start(wqkA[:, f, ts(half, 8), :],
                                  wqk_d[f, :, ts(half, 8), :])
            nc.sync.dma_start(wqkA[:, 0, 0:4, :], wqk_d[0, :, 0:4, :])
            nc.sync.dma_start(xt[:, 0, 0:512], x_d[:, 0, 0:512])
            nc.sync.dma_start(xt[:, 0, 512:1024], x_d[:, 0, 512:1024])
            nc.sync.dma_start(wqkA[:, 0, 4:8, :], wqk_d[0, :, 4:8, :])
            dma_wA(1, 0)
            nc.sync.dma_start(xt[:, 1], x_d[:, 1])
            nc.sync.dma_start(xt[:, 2], x_d[:, 2])
            dma_wA(2, 0)
            nc.sync.dma_start(xt[:, 3], x_d[:, 3])
            nc.sync.dma_start(cc[:], cs_d[0])
            nc.sync.dma_start(xt[:, 4], x_d[:, 4])
            dma_wA(3, 0)
            nc.sync.dma_start(xt[:, 5], x_d[:, 5])
            dma_wA(0, 1)
            nc.sync.dma_start(xt[:, 6], x_d[:, 6])
            dma_wA(1, 1)
            nc.sync.dma_start(xt[:, 7], x_d[:, 7])
            nc.sync.dma_start(ss[:], cs_d[1])
            nc.sync.dma_start(xt[:, 8], x_d[:, 8])
            dma_wA(2, 1)
            nc.sync.dma_start(xt[:, 9], x_d[:, 9])
            dma_wA(3, 1)
            for ci in range(10, NCC):
                nc.sync.dma_start(xt[:, ci], x_d[:, ci])
            wqkB = []
            for f in range(4):
                wt = wqkBp.tile([128, NCC, 128], BF16, tag="wqkB",
                                name=f"wqkB{f}")
                nc.sync.dma_start(wt[:], wqk_d[4 + f])
                wqkB.append(wt)
            nc.sync.dma_start(wvt[:], wv_d[:])
            nc.sync.dma_start(kxl[:], kxl_d[:])
            nc.sync.dma_start(vxl[:], vxl_d[:])
            nc.sync.dma_start(wpall[:], wproj_d[:])

            # preload the Exp activation table during the x stream
            warm = smallp.tile([1, 16], BF16, tag="warm")
            nc.scalar.activation(warm[:], ones[0:1, 0:16], AF.Exp, scale=1.0)

            def rope(psrc, fslot, tbsl):
                """qk[:, fslot, tbsl] = psrc*[cos;cos] + swap(psrc)*[-sin;sin]
                (swap via ACT rebase copies; tensor ops need same bases)."""
                sw = swp.tile([128, 512], F32, tag="sw")
                nc.scalar.copy(sw[0:64, :], psrc[64:128, :])
                nc.scalar.copy(sw[64:128, :], psrc[0:64, :])
                dst = qk[:, fslot, tbsl]
                t2 = t2p.tile([128, 512], BF16, tag="t2")
                nc.vector.tensor_mul(dst, psrc[:], cc[:, tbsl])
                nc.vector.tensor_mul(t2[:], sw[:], ss[:, tbsl])
                nc.vector.tensor_add(dst, dst, t2[:])

            # ---- phase A: q-GEMMs (f0-3 x tb0/tb1 minus (3,tb1)), ci-
            # wavefront over the streaming x so PE starts as soon as chunk 0
            # + w f0 land. Only 7 accumulators so the scp ring keeps a free
            # slot and the follow-on GEMMs never wait on the rope drain. ----
            qp = {}
            for fi in range(4):
                for tb in range(2):
                    if (fi, tb) == (3, 1):
                        continue
                    pool = pyp if fi < 2 else scp
                    qp[fi, tb] = pool.tile([128, 512], F32,
                                           tag="py" if fi < 2 else "sc",
                                           name=f"qA{fi}{tb}")
            skew = {0: 0, 1: 1, 2: 2, 3: 8}
            for w in range(NCC + 8):
                for fi in range(4):
                    ci = w - skew[fi]
                    if not (0 <= ci < NCC):
                        continue
                    for tb in range(2):
                        if (fi, tb) == (3, 1):
                            continue
                        nc.tensor.matmul(qp[fi, tb][:], wqkA[:, fi, ci, :],
                                         xt[:, ci, ts(tb, 512)],
                                         start=(ci == 0), stop=(ci == NCC - 1))

            # ---- phase A2: remaining q-GEMM, k-GEMMs (f4-7), v chunks 0-3,
            # with ropes interleaved so each scp ring slot is freed (rope
            # read) at least 4 allocations before it is reused ----
            def gemm_qk(wt_ap, tb, name):
                pk = scp.tile([128, 512], F32, tag="sc", name=name)
                for ci in range(NCC):
                    nc.tensor.matmul(pk[:], wt_ap[:, ci, :],
                                     xt[:, ci, ts(tb, 512)],
                                     start=(ci == 0), stop=(ci == NCC - 1))
                return pk

            def emit_v(tt, use_act):
                pv = scp.tile([128, 512], F32, tag="sc", name=f"pv{tt}")
                for ci in range(NCC):
                    nc.tensor.matmul(pv[:], xt[:, ci, ts(tt, 128)],
                                     wvt[:, ci, :],
                                     start=(ci == 0), stop=(ci == NCC - 1))
                if use_act:
                    nc.scalar.copy(vsb[:, tt, :], pv[:])
                else:
                    nc.vector.tensor_copy(vsb[:, tt, :], pv[:])

            rope(qp[2, 0], 2, ts(0, 512))
            pk31 = gemm_qk(wqkA[:, 3], "qA31")
            rope(qp[2, 1], 2, ts(1, 512))
            rope(qp[3, 0], 3, ts(0, 512))
            pk = {}
            pk[4, 0] = gemm_qk(wqkB[0], "k40")
            rope(pk31, 3, ts(1, 512))
            pk[4, 1] = gemm_qk(wqkB[0], "k41")
            rope(pk[4, 0], 4, ts(0, 512))
            pk[5, 0] = gemm_qk(wqkB[1], "k50")
            rope(pk[4, 1], 4, ts(1, 512))
            pk[5, 1] = gemm_qk(wqkB[1], "k51")
            rope(pk[5, 0], 5, ts(0, 512))
            pk[6, 0] = gemm_qk(wqkB[2], "k60")
            rope(pk[5, 1], 5, ts(1, 512))
            pk[6, 1] = gemm_qk(wqkB[2], "k61")
            rope(pk[6, 0], 6, ts(0, 512))
            pk[7, 0] = gemm_qk(wqkB[3], "k70")
            rope(pk[6, 1], 6, ts(1, 512))
            pk[7, 1] = gemm_qk(wqkB[3], "k71")
            rope(pk[7, 0], 7, ts(0, 512))
            emit_v(0, use_act=True)
            rope(pk[7, 1], 7, ts(1, 512))
            emit_v(1, use_act=True)
            rope(qp[0, 0], 0, ts(0, 512))
            rope(qp[0, 1], 0, ts(1, 512))
            emit_v(2, use_act=True)
            rope(qp[1, 0], 1, ts(0, 512))
            rope(qp[1, 1], 1, ts(1, 512))
            emit_v(3, use_act=True)

            # ---- attention quads + interleaved den / projection ----
            OFF = {0: 0, 1: 0, 2: 2, 3: 2}   # head slot stagger

            def emit_quad(tb, fillers, spacing=1, fill_from=1, den_cb=None):
                """Chunk-interleaved attention for 4 heads over 16 k-chunks.
                Heads 2,3 run two slots behind heads 0,1 so this quad's den
                work for heads 0,1 (den_cb stages "s1a"/"s3a") overlaps the
                staggered tail instead of serializing at the quad boundary.
                fillers are PE work callables popped per slot."""
                tbsl = ts(tb, 512)
                acc, py = {}, {}
                for h in range(4):
                    py[h] = pyp.tile([128, 512], F32, tag="py",
                                     name=f"py{tb}{h}")
                    acc[h] = accp.tile([128, 512], BF16, tag="acc",
                                       name=f"acc{tb}{h}")
                fill = list(fillers)
                pend = {}      # h -> (pt, lv, kc); av deferred by one chunk

                def emit_av(h):
                    pt_, lv_, kc_ = pend.pop(h)
                    nc.tensor.matmul(py[h][:], lv_, pt_[:],
                                     start=(kc_ == 0), stop=(kc_ == 15))
                for s in range(18):
                    for h in range(4):
                        kc = s - OFF[h]
                        if kc == 16:
                            emit_av(h)
                        if not (0 <= kc < 16):
                            continue
                        if kc < 8:
                            lk = kxl[:, h, ts(kc, 128)]
                            lv = vxl[:, kc, ts(h, 128)]
                        else:
                            lk = qk[:, 4 + h, ts(kc - 8, 128)]
                            lv = vsb[:, kc - 8, ts(h, 128)]
                        pss = scp.tile([128, 512], F32, tag="sc")
                        nc.tensor.matmul(pss[:], lk, qk[:, h, tbsl],
                                         start=True, stop=True)
                        pt = ptp.tile([128, 512], BF16, tag="pt")
                        nc.scalar.activation(pt[:], pss[:], AF.Exp,
                                             scale=SCALE)
                        if kc == 0:
                            nc.vector.tensor_copy(acc[h][:], pt[:])
                        else:
                            nc.vector.tensor_add(acc[h][:], acc[h][:], pt[:])
                        if h in pend:
                            emit_av(h)
                        pend[h] = (pt, lv, kc)
                    if s == 16 and den_cb is not None:
                        den_cb("s1a", acc, py)
                    if s == 17 and den_cb is not None:
                        den_cb("s3a", acc, py)
                    if fill and s >= fill_from and \
                            (s - fill_from) % spacing == 0:
                        fill.pop(0)()
                for h in range(4):
                    if h in pend:
                        emit_av(h)
                while fill:
                    fill.pop(0)()
                return acc, py

            def den_s1(acc, heads):
                """Per-head partition-sum of acc + reciprocal -> [1,512]."""
                recs = {}
                for h in heads:
                    pden = scp.tile([128, 512], F32, tag="sc", name="pden")
                    nc.tensor.matmul(pden[0:1, :], ones[:, 0:1],
                                     acc[h][:], start=True, stop=True)
                    rec = smallp.tile([1, 512], BF16, tag="rec")
                    nc.vector.reciprocal(rec[:], pden[0:1, :])
                    recs[h] = rec
                return recs

            def den_s3(recs, py, tb, heads):
                tbsl = ts(tb, 512)
                for h in heads:
                    pbc = scp.tile([128, 512], F32, tag="sc", name="pbc")
                    nc.tensor.matmul(pbc[:], ones[0:1, :], recs[h][:],
                                     start=True, stop=True)
                    rbc = smallp.tile([128, 512], F32, tag="rbc")
                    nc.vector.tensor_copy(rbc[:], pbc[:])
                    nc.vector.tensor_mul(ysb[:, h, tbsl], py[h][:], rbc[:])

            def emit_proj(ob, tb, pool=None, stage=None, parity=[0]):
                pool = pool or scp
                po = pool.tile([128, 512], F32,
                               tag="sc" if pool is scp else "py",
                               name=f"po{ob}{tb}")
                for yc in range(4):
                    nc.tensor.matmul(po[:], wpall[:, ob, yc, :],
                                     ysb[:, yc, ts(tb, 512)],
                                     start=(yc == 0), stop=(yc == 3))
                if stage is None:
                    ot = outp.tile([128, 512], BF16, tag="ot")
                    dst = ot[:]
                else:
                    st, j = stage
                    dst = st[:, j, :]
                if parity[0] % 2:
                    nc.scalar.copy(dst, po[:])
                else:
                    nc.vector.tensor_copy(dst, po[:])
                parity[0] += 1
                if stage is None:
                    nc.sync.dma_start(out_d[tb, :, ob, :], ot[:])

            # quad tb0: fill with v chunks 4-7 (each ready well before
            # its first AV read); den for heads 0,1 overlaps the stagger tail
            hold = {}

            def den0_cb(stage, acc, py):
                if stage == "s1a":
                    hold["r01"] = den_s1(acc, (0, 1))
                else:
                    den_s3(hold.pop("r01"), py, 0, (0, 1))

            acc_d, py_d = {}, {}
            acc_d[0], py_d[0] = emit_quad(
                0, [lambda tt=tt: emit_v(tt, use_act=False)
                    for tt in range(4, 8)], spacing=3, fill_from=4,
                den_cb=den0_cb)

            # tb0 heads 2,3 den + tb0 projections fill quad tb1
            def f_s1b():
                hold["r23"] = den_s1(acc_d[0], (2, 3))

            def f_s3b():
                den_s3(hold.pop("r23"), py_d[0], 0, (2, 3))

            def den1_cb(stage, acc, py):
                if stage == "s1a":
                    hold["r01b"] = den_s1(acc, (0, 1))
                else:
                    den_s3(hold.pop("r01b"), py, 1, (0, 1))

            # f_s3b must precede the first tb0 projection filler: projs
            # read all four ysb head rows, and heads 2,3 are written by s3b
            q1_fill = [f_s1b, f_s3b]
            q1_fill += [lambda ob=ob: emit_proj(ob, 0) for ob in range(12)]
            acc_d[1], py_d[1] = emit_quad(1, q1_fill, fill_from=0,
                                          den_cb=den1_cb)

            # tail: tb1 heads 2,3 den + remaining projections (w_proj
            # resident); po tiles alternate between both PSUM rings so the
            # ring-slot turnaround (mm -> sem -> copy -> free) stays off PE
            rec1 = den_s1(acc_d[1], (2, 3))
            for ob in range(12, 14):
                emit_proj(ob, 0)
            den_s3(rec1, py_d[1], 1, (2, 3))
            emit_proj(14, 0, pool=scp)
            emit_proj(15, 0, pool=pyp)
            for g in range(3):
                st = out4p.tile([128, 4, 512], BF16, tag="ot4",
                               name=f"st{g}")
                for j in range(4):
                    ob = 4 * g + j
                    emit_proj(ob, 1, pool=(scp if j % 2 == 0 else pyp),
                              stage=(st, j))
                nc.sync.dma_start(out_d[1, :, ts(g, 4), :], st[:])
            for ob in range(12, 16):
                emit_proj(ob, 1, pool=(scp if ob % 2 == 0 else pyp))

    nc.compile()
    return nc


def _get_nc():
    if "nc" not in _CACHE:
        _CACHE["nc"] = _build_nc()
    return _CACHE["nc"]


_PERM = np.concatenate([np.arange(0, HD, 2), np.arange(1, HD, 2)])
_PP = np.concatenate([_PERM + i * HD for i in range(HPC)])  # per-head-block


def make_in_maps(x, cos, sin, k_xl, v_xl, pos_emb, w_qkv, w_proj):
    """Host-side shard + layout prep: one input dict per core."""
    import ml_dtypes
    bf16 = ml_dtypes.bfloat16
    x = np.asarray(x, np.float32)
    cos = np.asarray(cos, np.float32)
    sin = np.asarray(sin, np.float32)
    k_xl = np.asarray(k_xl, np.float32) + np.asarray(pos_emb, np.float32)
    v_xl = np.asarray(v_xl, np.float32)
    w_qkv = np.asarray(w_qkv, np.float32)
    w_proj = np.asarray(w_proj, np.float32)

    # cs[0] = [cos; cos] ; cs[1] = [-sin; +sin]  (packed-rope factors)
    cs = np.ascontiguousarray(np.stack([
        np.concatenate([cos.T, cos.T], axis=0),
        np.concatenate([-sin.T, sin.T], axis=0),
    ]))

    in_maps = []
    for c in range(NCORES):
        b, g = c // CPB, c % CPB
        h0 = g * HPC
        cols = slice(h0 * HD, (h0 + HPC) * HD)

        # x: [pi, ci, t]
        x_arr = np.ascontiguousarray(
            x[b].T.reshape(NCC, 128, T).transpose(1, 0, 2)).astype(bf16)
        # w_q/w_k rows for this head group, rope-permuted; [f, pi, ci, fcol]
        wq = w_qkv[0 * D + h0 * HD:0 * D + (h0 + HPC) * HD][_PP]
        wk = w_qkv[1 * D + h0 * HD:1 * D + (h0 + HPC) * HD][_PP]
        wqk_rows = np.concatenate([wq, wk], axis=0)  # [1024, D]
        wqk_arr = np.ascontiguousarray(
            wqk_rows.reshape(8, 128, NCC, 128).transpose(0, 3, 2, 1)
        ).astype(bf16)
        # w_v rows (unpermuted); [pi, ci, col]
        wv_rows = w_qkv[2 * D + h0 * HD:2 * D + (h0 + HPC) * HD]  # [512, D]
        wv_arr = np.ascontiguousarray(
            wv_rows.T.reshape(NCC, 128, 512).transpose(1, 0, 2)).astype(bf16)
        # k_xl (pos already added): permuted cols, transposed; [pi, j, t]
        kxlT = k_xl[b][:, cols][:, _PP].T  # [512, XL]
        kxl_arr = np.ascontiguousarray(
            kxlT.reshape(4, 128, XL).transpose(1, 0, 2)).astype(bf16)
        # v_xl natural; [pi, j, col]
        vxl_arr = np.ascontiguousarray(
            v_xl[b][:, cols].reshape(8, 128, 512).transpose(1, 0, 2)
        ).astype(bf16)
        # w_proj column block, transposed; [pi, ob, yc, ocol]
        wprojT = w_proj[:, cols].T  # [512, D]
        wproj_arr = np.ascontiguousarray(
            wprojT.reshape(4, 128, 16, 128).transpose(1, 2, 0, 3)
        ).astype(bf16)

        in_maps.append({
            "x": x_arr, "wqk": wqk_arr, "wv": wv_arr, "cs": cs,
            "kxl": kxl_arr, "vxl": vxl_arr, "wproj": wproj_arr,
        })
    return in_maps


def unshard(results):
    """results: list of 8 dicts with 'out' [2, 128, 16, 512] (tb, pi, ob,
    col) -> [B, T, D]."""
    out = np.zeros((B, T, D), np.float32)
    for c in range(NCORES):
        b = c // CPB
        outT = np.asarray(results[c]["out"]).astype(np.float32)\
            .transpose(2, 1, 0, 3).reshape(D, T)
        out[b] += outT.T
    return out


def _get_runner():
    """Persistent jitted 8-core executable (avoids per-call retrace of the
    bass2jax lowering; the NEFF itself is cached by neuronx-cc)."""
    if "runner" in _CACHE:
        return _CACHE["runner"]
    import jax
    import jax.numpy as jnp
    from jax.sharding import Mesh, PartitionSpec, NamedSharding
    from jax.experimental.shard_map import shard_map
    from concourse.bass2jax import (_bass_exec_p, partition_id_tensor,
                                    install_neuronx_cc_hook)

    nc = _get_nc()
    install_neuronx_cc_hook()
    in_names, out_names, out_avals, zero_shapes = [], [], [], []
    for alloc in nc.m.functions[0].allocations:
        if not isinstance(alloc, mybir.MemoryLocationSet):
            continue
        name = alloc.memorylocations[0].name
        if alloc.kind == "ExternalInput":
            if nc.partition_id_tensor is None or \
                    name != nc.partition_id_tensor.name:
                in_names.append(name)
        elif alloc.kind == "ExternalOutput":
            shape = tuple(alloc.tensor_shape)
            np_dt = mybir.dt.np(alloc.dtype)
            out_names.append(name)
            out_avals.append(jax.core.ShapedArray(shape, np_dt))
            zero_shapes.append((shape, np_dt))
    n_params, n_outs = len(in_names), len(out_names)
    all_in = in_names + out_names
    if nc.partition_id_tensor is not None:
        all_in = all_in + [nc.partition_id_tensor.name]

    def _body(*args):
        operands = list(args)
        if nc.partition_id_tensor is not None:
            operands.append(partition_id_tensor())
        return tuple(_bass_exec_p.bind(
            *operands, out_avals=tuple(out_avals), in_names=tuple(all_in),
            out_names=tuple(out_names), lowering_input_output_aliases=(),
            sim_require_finite=True, sim_require_nnan=True, nc=nc))

    devices = jax.devices()[:NCORES]
    mesh = Mesh(np.asarray(devices), ("core",))
    fn = jax.jit(
        shard_map(_body, mesh=mesh,
                  in_specs=(PartitionSpec("core"),) * (n_params + n_outs),
                  out_specs=(PartitionSpec("core"),) * n_outs,
                  check_rep=False),
        donate_argnums=tuple(range(n_params, n_params + n_outs)),
        keep_unused=True)
    sharding = NamedSharding(mesh, PartitionSpec("core"))
    zfn = jax.jit(
        lambda: tuple(jnp.zeros((NCORES * s[0], *s[1:]), d)
                      for s, d in zero_shapes),
        out_shardings=(sharding,) * n_outs)
    runner = (fn, zfn, in_names, out_names, out_avals, sharding)
    _CACHE["runner"] = runner
    return runner


def kernel(x, cos, sin, k_xl, v_xl, pos_emb, w_qkv, w_proj, is_causal=0,
           **_ignored):
    import jax
    in_maps = make_in_maps(x, cos, sin, k_xl, v_xl, pos_emb, w_qkv, w_proj)
    fn, zfn, in_names, out_names, out_avals, sharding = _get_runner()
    concat_in = [
        jax.device_put(
            np.concatenate([in_maps[c][nm] for c in range(NCORES)], axis=0),
            sharding)
        for nm in in_names]
    outs = fn(*concat_in, *zfn())
    results = [
        {nm: np.asarray(outs[i]).reshape(NCORES, *out_avals[i].shape)[c]
         for i, nm in enumerate(out_names)}
        for c in range(NCORES)]
    _CACHE["last_results"] = None
    return unshard(results)
